# revision 28
# baseline (speedup 1.0000x reference)
"""Trainium2 Bass kernel for a 3-layer TransformerConv GNN encoder.

Contract: kernel(**inputs) takes the FULL inputs (node_features, edge_index,
edge_attr, batch, params, ln_gamma, ln_beta) and returns the FULL [64, 128]
output, distributing work across 8 NeuronCores internally.

Sharding: destination-node-range partitioning. Core c owns nodes
[c*12500, (c+1)*12500) and every edge whose dst lands there, bucketed by
128-node block. Softmax + aggregation are then fully core-local; layers
exchange activations with one AllGather of feature-major block tiles.
"""

import math
from contextlib import ExitStack

import numpy as np

import concourse.bass as bass
import concourse.mybir as mybir
import concourse.tile as tile
from concourse.bass import ds

F32 = mybir.dt.float32
I32 = mybir.dt.int32
AF = mybir.ActivationFunctionType
ALU = mybir.AluOpType


class Cfg:
    def __init__(self, n_cores, n_nodes, npc, in_dim, hidden, heads, n_graphs,
                 gb=3, loop_mode="python"):
        assert npc * n_cores >= n_nodes
        self.n_cores = n_cores
        self.n_nodes = n_nodes          # real nodes
        self.npc = npc                  # real nodes per core
        self.in_dim = in_dim
        self.hidden = hidden
        self.heads = heads
        self.n_graphs = n_graphs
        self.nblk = math.ceil(npc / 128)        # node blocks per core
        self.npad = self.nblk * 128             # padded nodes per core
        self.ntot = self.npad * n_cores         # padded node total
        self.nbt = self.nblk * n_cores          # total node blocks
        self.gb = gb                            # edge tiles batched per group
        self.loop_mode = loop_mode
        # layer dims: (H, D) per layer
        self.hd = [(heads, hidden // heads), (heads, hidden // heads), (1, hidden)]
        self.ln_eps = 1e-5


# ----------------------------------------------------------------------------
# Host-side preparation
# ----------------------------------------------------------------------------

def prep_weights(cfg, params, ln_gamma, ln_beta):
    """Fold weights into the device layout. Returns dict of replicated arrays."""
    w = {}
    in_c = cfg.in_dim
    hid = cfg.hidden
    for li, (p, (H, D)) in enumerate(zip(params, cfg.hd)):
        s = 1.0 / math.sqrt(D)
        Wq = np.asarray(p['Wq'], np.float32) * s
        bq = np.asarray(p['bq'], np.float32) * s
        We_row = np.asarray(p['We'], np.float32)[0]          # [hid]
        # q~ . We per head, folded into the node matmul
        Wqe = (Wq.reshape(in_c, H, D) * We_row.reshape(H, D)).sum(-1)   # [in_c, H]
        bqe = (bq.reshape(H, D) * We_row.reshape(H, D)).sum(-1)         # [H]
        Wskip = np.asarray(p['Wskip'], np.float32)
        bskip = np.asarray(p['bskip'], np.float32)
        w[f'wqx{li}'] = np.ascontiguousarray(
            np.concatenate([Wq, Wqe, Wskip], axis=1))        # [in_c, hid+H+hid]
        w[f'bqx{li}'] = np.concatenate([bq, bqe, bskip])[None, :].astype(np.float32)
        w[f'wkv{li}'] = np.ascontiguousarray(np.concatenate(
            [np.asarray(p['Wk'], np.float32), np.asarray(p['Wv'], np.float32)], axis=1))
        w[f'bkv{li}'] = np.concatenate(
            [np.asarray(p['bk'], np.float32), np.asarray(p['bv'], np.float32)])[None, :]
        w[f'we{li}'] = We_row[None, :].astype(np.float32)
        Wb = np.asarray(p['Wbeta'], np.float32)[:, 0]        # [3*hid]
        wA = Wb[:hid] + Wb[2 * hid:]
        wB = Wb[hid:2 * hid] - Wb[2 * hid:]
        w[f'wab{li}'] = np.concatenate([wA, wB])[None, :].astype(np.float32)
        in_c = hid
    w['gb_ln'] = np.concatenate([np.asarray(ln_gamma, np.float32),
                                 np.asarray(ln_beta, np.float32)])[None, :]
    return w


def prep_edges(cfg, edge_index, edge_attr):
    """Bucket edges by (dst core, dst block); pad to uniform T tiles of 128."""
    src = np.asarray(edge_index[0]).astype(np.int64)
    dst = np.asarray(edge_index[1]).astype(np.int64)
    attr = np.asarray(edge_attr, np.float32).reshape(-1)
    E = src.shape[0]
    npc, nblk, npad = cfg.npc, cfg.nblk, cfg.npad

    src_pad = (src + (src // npc) * (npad - npc)).astype(np.int64)
    core = dst // npc
    dst_loc = dst - core * npc
    blk = dst_loc // 128
    dst_in_blk = (dst_loc % 128).astype(np.float32)

    key = core * nblk + blk
    order = np.argsort(key, kind='stable')
    key_s = key[order]
    counts = np.bincount(key, minlength=cfg.n_cores * nblk)
    T = max(1, math.ceil(counts.max() / 128))
    starts = np.cumsum(counts) - counts
    rank = np.arange(E, dtype=np.int64) - starts[key_s]
    slot_p = (rank % 128).astype(np.int64)
    slot_t = (rank // 128).astype(np.int64)

    per_core = []
    for c in range(cfg.n_cores):
        srcs = np.zeros((nblk, 128, T), np.int32)
        dsts = np.full((nblk, 128, T), -1.0, np.float32)
        attrs = np.zeros((nblk, 128, T), np.float32)
        m = (key_s // nblk) == c
        b = (key_s[m] - c * nblk)
        p = slot_p[m]
        t = slot_t[m]
        srcs[b, p, t] = src_pad[order[m]]
        dsts[b, p, t] = dst_in_blk[order[m]]
        attrs[b, p, t] = attr[order[m]]
        per_core.append({'srcs': srcs, 'dsts': dsts, 'attrs': attrs})
    return per_core, T


def prep_nodes(cfg, node_features, batch):
    x = np.asarray(node_features, np.float32)
    in_dim = x.shape[1]
    xp = np.zeros((cfg.ntot, in_dim), np.float32)
    bp = np.full((cfg.n_cores, cfg.npad), -1.0, np.float32)
    b = np.asarray(batch)
    for c in range(cfg.n_cores):
        lo = c * cfg.npc
        hi = min((c + 1) * cfg.npc, cfg.n_nodes)
        n = hi - lo
        if n > 0:
            xp[c * cfg.npad:c * cfg.npad + n] = x[lo:hi]
            bp[c, :n] = b[lo:hi]
    # feature-major block tiles: [nbt, in_dim, 128]
    xt_all = np.ascontiguousarray(
        xp.reshape(cfg.nbt, 128, in_dim).transpose(0, 2, 1))
    batch_t = bp.reshape(cfg.n_cores, cfg.nblk, 128, 1)
    return xt_all, batch_t


# ----------------------------------------------------------------------------
# Device program
# ----------------------------------------------------------------------------

def build_program(cfg, T):
    from concourse import bacc
    nc = bacc.Bacc()
    hid = cfg.hidden
    NBLK, NBT, NTOT = cfg.nblk, cfg.nbt, cfg.ntot
    GB = cfg.gb
    n_groups = math.ceil(T / GB)
    # group sizes (last may be smaller)
    gsizes = [min(GB, T - g * GB) for g in range(n_groups)]

    # ---- I/O ----
    xt0_all = nc.declare_dram_parameter("xt0_all", [NBT, cfg.in_dim, 128], F32, isOutput=False)
    xt0_own = nc.declare_dram_parameter("xt0_own", [NBLK, cfg.in_dim, 128], F32, isOutput=False)
    srcs = nc.declare_dram_parameter("srcs", [NBLK, 128, T], I32, isOutput=False)
    dsts = nc.declare_dram_parameter("dsts", [NBLK, 128, T], F32, isOutput=False)
    attrs = nc.declare_dram_parameter("attrs", [NBLK, 128, T], F32, isOutput=False)
    batch_t = nc.declare_dram_parameter("batchp", [NBLK, 128, 1], F32, isOutput=False)
    wq_ext, bq_ext, wkv_ext, bkv_ext, we_ext, wab_ext = [], [], [], [], [], []
    in_c = cfg.in_dim
    LQs = []
    for li, (H, D) in enumerate(cfg.hd):
        LQ = hid + H
        LQs.append(LQ)
        wq_ext.append(nc.declare_dram_parameter(f"wqx{li}", [in_c, LQ + hid], F32, isOutput=False))
        bq_ext.append(nc.declare_dram_parameter(f"bqx{li}", [1, LQ + hid], F32, isOutput=False))
        wkv_ext.append(nc.declare_dram_parameter(f"wkv{li}", [in_c, 2 * hid], F32, isOutput=False))
        bkv_ext.append(nc.declare_dram_parameter(f"bkv{li}", [1, 2 * hid], F32, isOutput=False))
        we_ext.append(nc.declare_dram_parameter(f"we{li}", [1, hid], F32, isOutput=False))
        wab_ext.append(nc.declare_dram_parameter(f"wab{li}", [1, 2 * hid], F32, isOutput=False))
        in_c = hid
    gb_ext = nc.declare_dram_parameter("gb_ln", [1, 2 * hid], F32, isOutput=False)
    out_ext = nc.declare_dram_parameter("pool_out", [cfg.n_graphs, hid + 1], F32, isOutput=True)

    # ---- internal DRAM ----
    kv_l = [nc.dram_tensor(f"kv{li}", [NTOT, 2 * hid], F32) for li in range(3)]
    qx_l = [nc.dram_tensor(f"qx{li}", [NBLK, 128, LQs[li] + hid], F32) for li in range(3)]
    own_l = [None, nc.dram_tensor("own1", [NBLK, hid, 128], F32),
             nc.dram_tensor("own2", [NBLK, hid, 128], F32)]
    ag_in = [nc.dram_tensor(f"agin{li}", [NBLK * hid, 128], F32) for li in range(2)]
    ag_out = [nc.dram_tensor(f"agout{li}", [NBT * hid, 128], F32) for li in range(2)]

    with tile.TileContext(nc) as tc, ExitStack() as ctx:
        cpool = ctx.enter_context(tc.tile_pool(name="const", bufs=1))
        wpool = ctx.enter_context(tc.tile_pool(name="wts", bufs=1))
        npool = ctx.enter_context(tc.tile_pool(name="nodeio", bufs=3))
        npsum = ctx.enter_context(tc.tile_pool(name="npsum", bufs=2, space="PSUM"))
        epool = ctx.enter_context(tc.tile_pool(name="edge", bufs=2))
        gpool = ctx.enter_context(tc.tile_pool(name="gath", bufs=2))
        tpool = ctx.enter_context(tc.tile_pool(name="etmp", bufs=2))
        epsum = ctx.enter_context(tc.tile_pool(name="epsum", bufs=2, space="PSUM"))
        qpsum = ctx.enter_context(tc.tile_pool(name="qpsum", bufs=3, space="PSUM"))
        bpsum = ctx.enter_context(tc.tile_pool(name="bpsum", bufs=1, space="PSUM"))

        # ---- constants ----
        iota_p_i = cpool.tile([128, 1], dtype=I32, tag="iopi")
        nc.gpsimd.iota(iota_p_i[:], pattern=[[0, 1]], base=0, channel_multiplier=1)
        iota_p = cpool.tile([128, 1], dtype=F32, tag="iop")
        nc.vector.tensor_copy(iota_p[:], iota_p_i[:])
        iota_f_i = cpool.tile([128, GB * 128], dtype=I32, tag="iofi")
        nc.gpsimd.iota(iota_f_i[:], pattern=[[0, GB], [1, 128]], base=0, channel_multiplier=0)
        iota_f = cpool.tile([128, GB * 128], dtype=F32, tag="iof")
        nc.vector.tensor_copy(iota_f[:], iota_f_i[:])
        iota_g_i = cpool.tile([128, cfg.n_graphs], dtype=I32, tag="iogi")
        nc.gpsimd.iota(iota_g_i[:], pattern=[[1, cfg.n_graphs]], base=0, channel_multiplier=0)
        iota_g = cpool.tile([128, cfg.n_graphs], dtype=F32, tag="iog")
        nc.vector.tensor_copy(iota_g[:], iota_g_i[:])
        ident = cpool.tile([128, 128], dtype=F32, tag="ident")
        from concourse.masks import make_identity
        make_identity(nc, ident[:])

        # replicated per-layer consts: [1,W] row -> [128,W] via K=1 outer product
        ones_row = cpool.tile([1, 128], dtype=F32, tag="ones1")
        nc.vector.memset(ones_row[:], 1.0)

        def load_rep(ext, width, tag):
            row = cpool.tile([1, width], dtype=F32, tag=tag + "r")
            nc.sync.dma_start(row[:], ext[:])
            ps = bpsum.tile([128, width], dtype=F32, tag="pps")
            nc.tensor.matmul(ps[:], ones_row[:1, :], row[:1, :], start=True, stop=True)
            rep = cpool.tile([128, width], dtype=F32, tag=tag)
            nc.vector.tensor_copy(rep[:], ps[:])
            return rep

        bqx_rep = [load_rep(bq_ext[li], LQs[li] + hid, f"bqx{li}") for li in range(3)]
        bkv_rep = [load_rep(bkv_ext[li], 2 * hid, f"bkv{li}") for li in range(3)]
        we_rep = [load_rep(we_ext[li], hid, f"we{li}") for li in range(3)]
        wab_rep = [load_rep(wab_ext[li], 2 * hid, f"wab{li}") for li in range(3)]
        gb_rep = load_rep(gb_ext, 2 * hid, "gbln")

        pool_acc = cpool.tile([cfg.n_graphs, hid + 1], dtype=F32, tag="poolacc")
        nc.vector.memset(pool_acc[:], 0.0)
        eps_t = cpool.tile([128, 1], dtype=F32, tag="epst")
        nc.vector.memset(eps_t[:], cfg.ln_eps)

        in_c = cfg.in_dim
        for li, (H, D) in enumerate(cfg.hd):
            LQ = LQs[li]
            W2 = hid + 2 * H
            kv = kv_l[li]
            qx = qx_l[li]
            xt_blocks = xt0_all if li == 0 else ag_out[li - 1][:, :].rearrange(
                "(b f) n -> b f n", f=hid)
            xt_own = xt0_own if li == 0 else own_l[li]

            # weights to SBUF
            wqx_sb = wpool.tile([in_c, LQ + hid], dtype=F32, tag=f"wqx{li}")
            nc.sync.dma_start(wqx_sb[:], wq_ext[li][:])
            wkv_sb = wpool.tile([in_c, 2 * hid], dtype=F32, tag=f"wkv{li}")
            nc.sync.dma_start(wkv_sb[:], wkv_ext[li][:])

            # ---- phase N: kv table for ALL node blocks ----
            for j in range(NBT):
                lhsT = npool.tile([in_c, 128], dtype=F32, tag="lhsT")
                nc.sync.dma_start(lhsT[:], xt_blocks[j])
                ps = npsum.tile([128, 2 * hid], dtype=F32, tag="nps")
                nc.tensor.matmul(ps[:], lhsT[:], wkv_sb[:], start=True, stop=True)
                kvt = npool.tile([128, 2 * hid], dtype=F32, tag="kvt")
                nc.vector.tensor_tensor(out=kvt[:], in0=ps[:], in1=bkv_rep[li][:], op=ALU.add)
                nc.sync.dma_start(kv[ds(j * 128, 128), :], kvt[:])

            # ---- phase N: q|qWe|skip for OWN node blocks ----
            for j in range(NBLK):
                lhsT = npool.tile([in_c, 128], dtype=F32, tag="lhsT")
                nc.sync.dma_start(lhsT[:], xt_own[j])
                ps = npsum.tile([128, LQ + hid], dtype=F32, tag="nps")
                nc.tensor.matmul(ps[:], lhsT[:], wqx_sb[:], start=True, stop=True)
                qxt = npool.tile([128, LQ + hid], dtype=F32, tag="qxt")
                nc.vector.tensor_tensor(out=qxt[:], in0=ps[:], in1=bqx_rep[li][:], op=ALU.add)
                nc.sync.dma_start(qx[j], qxt[:])

            # ---- phase E: per-block edge processing ----
            def edge_block(i):
                qx_sb = epool.tile([128, LQ + hid], dtype=F32, tag="qxsb")
                nc.sync.dma_start(qx_sb[:], qx[ds(i, 1)])
                src_sb = epool.tile([128, T], dtype=I32, tag="srcsb")
                nc.sync.dma_start(src_sb[:], srcs[ds(i, 1)])
                dst_sb = epool.tile([128, T], dtype=F32, tag="dstsb")
                nc.sync.dma_start(dst_sb[:], dsts[ds(i, 1)])
                attr_sb = epool.tile([128, T], dtype=F32, tag="attrsb")
                nc.sync.dma_start(attr_sb[:], attrs[ds(i, 1)])
                kv_g = gpool.tile([128, T * 2 * hid], dtype=F32, tag="kvg")
                kv3 = kv_g[:].rearrange("p (t w) -> p t w", t=T)
                # one indirect DMA per 128-edge tile: HW requires [128,1] offsets
                for t in range(T):
                    nc.gpsimd.indirect_dma_start(
                        out=kv3[:, t, :], out_offset=None, in_=kv[:],
                        in_offset=bass.IndirectOffsetOnAxis(ap=src_sb[:, t:t + 1],
                                                            axis=0))
                psum_blk = epsum.tile([128, W2], dtype=F32, tag="psblk")
                first_mm = [True]
                for g in range(n_groups):
                    gs = gsizes[g]
                    sl = slice(g * GB, g * GB + gs)
                    oh_ne = tpool.tile([128, gs * 128], dtype=F32, tag="ohne")
                    for u in range(gs):
                        t = g * GB + u
                        dps = qpsum.tile([128, 128], dtype=F32, tag="qeps")
                        nc.tensor.transpose(
                            dps[:], dst_sb[:, t:t + 1].to_broadcast([128, 128]),
                            ident[:])
                        nc.vector.tensor_tensor(
                            out=oh_ne[:, u * 128:(u + 1) * 128],
                            in0=iota_p[:, :1].to_broadcast([128, 128]),
                            in1=dps[:], op=ALU.is_equal)
                    oh_en = tpool.tile([128, gs * 128], dtype=F32, tag="ohen")
                    nc.vector.tensor_tensor(
                        out=oh_en[:], in0=dst_sb[:, sl, None].to_broadcast([128, gs, 128]),
                        in1=iota_f[:, :gs * 128].rearrange("p (g n) -> p g n", g=gs),
                        op=ALU.is_equal)
                    qe_ps = qpsum.tile([128, gs * LQ], dtype=F32, tag="qeps")
                    for u in range(gs):
                        nc.tensor.matmul(qe_ps[:, u * LQ:(u + 1) * LQ],
                                         oh_ne[:, u * 128:(u + 1) * 128], qx_sb[:, :LQ],
                                         start=True, stop=True)
                    qe3 = qe_ps[:].rearrange("p (g l) -> p g l", g=gs)
                    qk = tpool.tile([128, gs * hid], dtype=F32, tag="qk")
                    qk3 = qk[:].rearrange("p (g f) -> p g f", g=gs)
                    nc.vector.tensor_tensor(out=qk3, in0=qe3[:, :, :hid],
                                            in1=kv3[:, sl, :hid], op=ALU.mult)
                    lg = tpool.tile([128, gs * H], dtype=F32, tag="lg")
                    nc.vector.tensor_reduce(
                        out=lg[:], in_=qk[:].rearrange("p (g h d) -> p g h d", g=gs, h=H),
                        axis=mybir.AxisListType.X, op=ALU.add)
                    att = tpool.tile([128, gs * H], dtype=F32, tag="att")
                    nc.vector.tensor_tensor(
                        out=att[:].rearrange("p (g h) -> p g h", g=gs),
                        in0=qe3[:, :, hid:LQ],
                        in1=attr_sb[:, sl, None].to_broadcast([128, gs, H]),
                        op=ALU.mult)
                    nc.vector.tensor_tensor(out=lg[:], in0=lg[:], in1=att[:], op=ALU.add)
                    rhs_cat = tpool.tile([128, gs * W2], dtype=F32, tag="rhscat")
                    rc3 = rhs_cat[:].rearrange("p (g w) -> p g w", g=gs)
                    nc.scalar.activation(rc3[:, :, hid:hid + H], lg[:], AF.Exp)
                    nc.vector.tensor_tensor(
                        out=rc3[:, :, hid + H:], in0=rc3[:, :, hid:hid + H],
                        in1=attr_sb[:, sl, None].to_broadcast([128, gs, H]),
                        op=ALU.mult)
                    nc.vector.tensor_tensor(
                        out=rc3[:, :, :hid].rearrange("p g (h d) -> p g h d", h=H),
                        in0=kv3[:, sl, hid:].rearrange("p g (h d) -> p g h d", h=H),
                        in1=rc3[:, :, hid:hid + H][:, :, :, None].to_broadcast(
                            [128, gs, H, D]),
                        op=ALU.mult)
                    for u in range(gs):
                        last = (g == n_groups - 1) and (u == gs - 1)
                        nc.tensor.matmul(psum_blk[:], oh_en[:, u * 128:(u + 1) * 128],
                                         rc3[:, u, :],
                                         start=first_mm[0], stop=last)
                        first_mm[0] = False

                # ---- block epilogue ----
                xr_sb = qx_sb[:, LQ:]
                denom = tpool.tile([128, H], dtype=F32, tag="denom")
                nc.vector.tensor_scalar_add(denom[:], psum_blk[:, hid:hid + H], 1e-16)
                recip = tpool.tile([128, H], dtype=F32, tag="recip")
                nc.vector.reciprocal(recip[:], denom[:])
                sH = tpool.tile([128, H], dtype=F32, tag="sH")
                nc.vector.tensor_tensor(out=sH[:], in0=psum_blk[:, hid + H:],
                                        in1=recip[:], op=ALU.mult)
                m1 = tpool.tile([128, hid], dtype=F32, tag="m1")
                nc.vector.tensor_tensor(
                    out=m1[:].rearrange("p (h d) -> p h d", h=H),
                    in0=psum_blk[:, :hid].rearrange("p (h d) -> p h d", h=H),
                    in1=recip[:, :, None].to_broadcast([128, H, D]), op=ALU.mult)
                msg = tpool.tile([128, hid], dtype=F32, tag="msg")
                nc.vector.tensor_tensor(
                    out=msg[:].rearrange("p (h d) -> p h d", h=H),
                    in0=we_rep[li][:].rearrange("p (h d) -> p h d", h=H),
                    in1=sH[:, :, None].to_broadcast([128, H, D]), op=ALU.mult)
                nc.vector.tensor_tensor(out=msg[:], in0=msg[:], in1=m1[:], op=ALU.add)
                # beta gate
                bt = tpool.tile([128, hid], dtype=F32, tag="btmp")
                nc.vector.tensor_tensor(out=bt[:], in0=msg[:], in1=wab_rep[li][:, :hid],
                                        op=ALU.mult)
                bl = tpool.tile([128, 2], dtype=F32, tag="bl")
                nc.vector.tensor_reduce(out=bl[:, :1], in_=bt[:],
                                        axis=mybir.AxisListType.X, op=ALU.add)
                nc.vector.tensor_tensor(out=bt[:], in0=xr_sb, in1=wab_rep[li][:, hid:],
                                        op=ALU.mult)
                nc.vector.tensor_reduce(out=bl[:, 1:], in_=bt[:],
                                        axis=mybir.AxisListType.X, op=ALU.add)
                blog = tpool.tile([128, 1], dtype=F32, tag="blog")
                nc.vector.tensor_tensor(out=blog[:], in0=bl[:, :1], in1=bl[:, 1:],
                                        op=ALU.add)
                beta_t = tpool.tile([128, 1], dtype=F32, tag="betat")
                nc.scalar.activation(beta_t[:], blog[:], AF.Sigmoid)
                dt_ = tpool.tile([128, hid], dtype=F32, tag="dt")
                nc.vector.tensor_tensor(out=dt_[:], in0=xr_sb, in1=msg[:], op=ALU.subtract)
                ob = tpool.tile([128, hid], dtype=F32, tag="ob")
                nc.vector.tensor_scalar(out=ob[:], in0=dt_[:], scalar1=beta_t[:, :1],
                                        scalar2=None, op0=ALU.mult)
                nc.vector.tensor_tensor(out=ob[:], in0=ob[:], in1=msg[:], op=ALU.add)

                if li < 2:
                    obr = tpool.tile([128, hid], dtype=F32, tag="obr")
                    nc.scalar.activation(obr[:], ob[:], AF.Relu)
                    trp = qpsum.tile([128, 128], dtype=F32, tag="qeps")
                    nc.tensor.transpose(trp[:], obr[:], ident[:])
                    trs = tpool.tile([128, 128], dtype=F32, tag="trs")
                    nc.vector.tensor_copy(trs[:], trp[:])
                    nc.sync.dma_start(
                        ag_in[li][:, :].rearrange("(b f) n -> b f n", f=hid)[ds(i, 1)],
                        trs[:])
                    nc.sync.dma_start(own_l[li + 1][ds(i, 1)], trs[:])
                else:
                    # LayerNorm + graph pooling
                    mn = tpool.tile([128, 1], dtype=F32, tag="mn")
                    nc.vector.tensor_reduce(out=mn[:], in_=ob[:],
                                            axis=mybir.AxisListType.X, op=ALU.add)
                    mns = tpool.tile([128, 1], dtype=F32, tag="mns")
                    nc.scalar.mul(mns[:], mn[:], 1.0 / hid)
                    xc = tpool.tile([128, hid], dtype=F32, tag="xc")
                    nc.vector.tensor_scalar(out=xc[:], in0=ob[:], scalar1=mns[:, :1],
                                            scalar2=None, op0=ALU.subtract)
                    sq = tpool.tile([128, hid], dtype=F32, tag="sq")
                    nc.vector.tensor_tensor(out=sq[:], in0=xc[:], in1=xc[:], op=ALU.mult)
                    vr = tpool.tile([128, 1], dtype=F32, tag="vr")
                    nc.vector.tensor_reduce(out=vr[:], in_=sq[:],
                                            axis=mybir.AxisListType.X, op=ALU.add)
                    sd = tpool.tile([128, 1], dtype=F32, tag="sd")
                    nc.scalar.activation(sd[:], vr[:], AF.Sqrt, bias=eps_t[:, :1],
                                         scale=1.0 / hid)
                    rstd = tpool.tile([128, 1], dtype=F32, tag="rstd")
                    nc.vector.reciprocal(rstd[:], sd[:])
                    xcat = tpool.tile([128, hid + 1], dtype=F32, tag="xcat")
                    nc.vector.tensor_scalar(out=xcat[:, :hid], in0=xc[:],
                                            scalar1=rstd[:, :1], scalar2=None,
                                            op0=ALU.mult)
                    nc.vector.tensor_tensor(out=xcat[:, :hid], in0=xcat[:, :hid],
                                            in1=gb_rep[:, :hid], op=ALU.mult)
                    nc.vector.tensor_tensor(out=xcat[:, :hid], in0=xcat[:, :hid],
                                            in1=gb_rep[:, hid:], op=ALU.add)
                    nc.vector.memset(xcat[:, hid:], 1.0)
                    bat_sb = epool.tile([128, 1], dtype=F32, tag="batsb")
                    nc.sync.dma_start(bat_sb[:], batch_t[ds(i, 1)])
                    oh_g = tpool.tile([128, cfg.n_graphs], dtype=F32, tag="ohg")
                    nc.vector.tensor_tensor(
                        out=oh_g[:], in0=bat_sb[:, :1].to_broadcast([128, cfg.n_graphs]),
                        in1=iota_g[:], op=ALU.is_equal)
                    pps = bpsum.tile([cfg.n_graphs, hid + 1], dtype=F32, tag="pps")
                    nc.tensor.matmul(pps[:], oh_g[:], xcat[:], start=True, stop=True)
                    nc.vector.tensor_tensor(out=pool_acc[:], in0=pool_acc[:], in1=pps[:],
                                            op=ALU.add)

            if cfg.loop_mode == "python":
                for i in range(NBLK):
                    edge_block(i)
            else:
                tc.For_i_unrolled(0, NBLK, 1, edge_block, max_unroll=7)

            # ---- inter-layer AllGather ----
            if li < 2:
                nc.gpsimd.collective_compute(
                    "AllGather", ALU.bypass,
                    replica_groups=[list(range(cfg.n_cores))],
                    ins=[ag_in[li][:, :]],
                    outs=[ag_out[li][:, :]],
                )
            in_c = hid

        outsb = cpool.tile([cfg.n_graphs, hid + 1], dtype=F32, tag="outsb")
        nc.vector.tensor_copy(outsb[:], pool_acc[:])
        nc.sync.dma_start(out_ext[:], outsb[:])

    nc.finalize()
    return nc


# ----------------------------------------------------------------------------
# Host entry points
# ----------------------------------------------------------------------------

def make_in_maps(cfg, node_features, edge_index, edge_attr, batch,
                 params, ln_gamma, ln_beta):
    w = prep_weights(cfg, params, ln_gamma, ln_beta)
    per_core_edges, T = prep_edges(cfg, edge_index, edge_attr)
    xt_all, batch_t = prep_nodes(cfg, node_features, batch)
    in_maps = []
    for c in range(cfg.n_cores):
        m = dict(w)
        m['xt0_all'] = xt_all
        m['xt0_own'] = np.ascontiguousarray(
            xt_all[c * cfg.nblk:(c + 1) * cfg.nblk])
        m['srcs'] = per_core_edges[c]['srcs']
        m['dsts'] = per_core_edges[c]['dsts']
        m['attrs'] = per_core_edges[c]['attrs']
        m['batchp'] = np.ascontiguousarray(batch_t[c])
        in_maps.append(m)
    return in_maps, T


def combine_outputs(cfg, outs):
    S = np.zeros((cfg.n_graphs, cfg.hidden + 1), np.float64)
    for o in outs:
        S += o['pool_out'].astype(np.float64)
    sums = S[:, :cfg.hidden]
    cnt = S[:, cfg.hidden:]
    res = np.where(cnt > 0, sums / np.maximum(cnt, 1.0), 0.0)
    return res.astype(np.float32)


LAST_RESULTS = None


def kernel(node_features, edge_index, edge_attr, batch, params,
           ln_gamma, ln_beta) -> np.ndarray:
    import os
    global LAST_RESULTS
    cfg = Cfg(n_cores=8, n_nodes=100000, npc=12500, in_dim=64, hidden=128,
              heads=8, n_graphs=64, gb=3,
              loop_mode=os.environ.get("GNN_LOOP_MODE", "python"))
    in_maps, T = make_in_maps(cfg, node_features, edge_index, edge_attr,
                              batch, params, ln_gamma, ln_beta)
    nc = build_program(cfg, T)
    from concourse.bass_utils import run_bass_kernel_spmd
    trace = bool(int(os.environ.get("GNN_TRACE", "0")))
    res = run_bass_kernel_spmd(nc, in_maps, core_ids=list(range(cfg.n_cores)),
                               trace=trace)
    LAST_RESULTS = res
    return combine_outputs(cfg, res.results)
